# revision 37
# baseline (speedup 1.0000x reference)
"""Trainium2 Bass kernel for nn_MultiHeadSelfTokenAttention.

Reference computation (per (b, s) slice, X = hidden[b, s] in [T=128, H=768]):
    q      = X @ Wq + bq                       [T, 12]     (per-token per-head logit)
    scores = q + mask[:, None] * (-10000)
    alpha  = softmax(scores, axis=T)           [T, 12]
    v      = (X @ Wv + bv).reshape(T, 12, 64)
    res    = einsum('th,thd->hd', alpha, v)    [12, 64] -> [768]
    out    = LN(res @ Wo + bo) * gamma + beta  [768]

Key algebraic restructure: with e = exp(scores) (unnormalized) and Z = sum_t e,
    Yu[head, h]  = sum_t e[t, head] * X[t, h]          (unnormalized pool)
    G[d, s]      = (Yu[head(d)] @ Wv[:, d]) + bv[d] * Z[s, head(d)]
    P[d, s]      = G[d, s] / Z[s, head(d)]
so V is never materialized AND the softmax normalization folds into the
G-route extract (one tensor_tensor with a partition-broadcast 1/Z).

v6 (on top of v5):
  - all bf16 operands (X, Wq, Wv, Wo, mask, ident, bv, bo) are pre-cast and
    pre-laid-out on the HOST: HBM traffic drops 17.4MB -> 8.8MB per core,
    every load runs on the two HWDGE rings (no SWDGE cast DMA, no on-chip
    weight cast passes), and the gpsimd engine only does the 1/Z broadcast.
  - X blocks land every ~5-7us so the PE never starves early and the HAM
    clock gate stays at 2.4 GHz.
  - ps_g double-buffered (G-route dc iterations pipeline); ett/z share the
    ps_xt slots.
  - g_route quarter per iteration right after stage_q (fills the exp
    round-trip); sqrt-table warm is data-dependent on the last exp so the
    scheduler cannot hoist it before.
  - LN tail: eps folded into Sqrt bias, xn split ACT/DVE, gamma/beta apply
    split DVE/gpsimd.

Sharding: data-parallel across batch; core b handles hidden_states[b]
(32 sents).  Weights replicated.  No collectives.
"""

import os
import sys
from contextlib import ExitStack

import numpy as np

for _p in ("/opt/trn_rl_repo", "/root/.axon_site/_ro/trn_rl_repo"):
    if os.path.isdir(_p) and _p not in sys.path:
        sys.path.insert(0, _p)

import ml_dtypes

import concourse.bacc as bacc
import concourse.bass as bass
import concourse.bass_utils as _bu
import concourse.tile as tile
from concourse import mybir
from concourse.bass_utils import run_bass_kernel_spmd



F32 = mybir.dt.float32
BF16 = mybir.dt.bfloat16
AF = mybir.ActivationFunctionType
ALU = mybir.AluOpType

HIDDEN = 768
HEADS = 12
B, S, T = 8, 32, 128
HC = HIDDEN // 128  # 6 chunks of the hidden dim
LN_EPS = 1e-5
MASK_NEG = -10000.0
N_CORES = 8
BS = 8  # sents per block
NBLK = S // BS


def build_kernel():
    nc = bacc.Bacc(trn_type="TRN2", target_bir_lowering=False, debug=False)

    # all-bf16 device inputs, pre-laid-out on the host
    hs = nc.dram_tensor("hs", [S, T, HIDDEN], BF16, kind="ExternalInput").ap()
    # identwq[128, 0:128] = I128, [128, 128:200] = Wq chunked (h = c*128 + p)
    identwq = nc.dram_tensor(
        "identwq", [128, 128 + HC * HEADS], BF16, kind="ExternalInput"
    ).ap()
    # rows[0, :]: mask*(-1e4) flattened (4096) | bv (768) | bo (768)
    rows = nc.dram_tensor(
        "rows", [1, S * T + 2 * HIDDEN], BF16, kind="ExternalInput"
    ).ap()
    # wvo[128, 0:4608] = Wv chunked, [128, 4608:9216] = Wo chunked
    wvo = nc.dram_tensor(
        "wvo", [128, 2 * HC * HIDDEN], BF16, kind="ExternalInput"
    ).ap()
    bq = nc.dram_tensor("bq", [HEADS], F32, kind="ExternalInput").ap()
    gb = nc.dram_tensor("gb", [2 * HIDDEN], F32, kind="ExternalInput").ap()
    out = nc.dram_tensor("out", [S, HIDDEN], F32, kind="ExternalOutput").ap()

    with tile.TileContext(nc) as tc:
        kernel_body(tc, out, hs, identwq, rows, wvo, bq, gb)
    nc.compile()
    return nc


def kernel_body(tc, out, hs, identwq, rows, wvo, bq, gb):
    nc = tc.nc
    with ExitStack() as ctx:
        consts = ctx.enter_context(tc.tile_pool(name="consts", bufs=1))
        xp = ctx.enter_context(tc.tile_pool(name="x", bufs=4))
        xtp = ctx.enter_context(tc.tile_pool(name="xt", bufs=4))
        smallp = ctx.enter_context(tc.tile_pool(name="small", bufs=2))
        psctx = ExitStack()
        ps_xt = psctx.enter_context(tc.tile_pool(name="ps_xt", bufs=2, space="PSUM"))
        ps_qt = psctx.enter_context(tc.tile_pool(name="ps_qt", bufs=1, space="PSUM"))
        ps_yt = psctx.enter_context(tc.tile_pool(name="ps_yt", bufs=2, space="PSUM"))
        ps_g = psctx.enter_context(tc.tile_pool(name="ps_g", bufs=2, space="PSUM"))

        # ---- tiles ----
        iw_sb = consts.tile([128, 128 + HC * HEADS], BF16, tag="iw")
        ident_sb = iw_sb[:, 0:128]
        wq_sb = iw_sb[:, 128 : 128 + HC * HEADS]
        rows_sb = consts.tile([1, S * T + 2 * HIDDEN], BF16, tag="rows")
        maskneg_row = rows_sb[:, 0 : S * T]  # already scaled by -1e4 on host
        bv_row = rows_sb[:, S * T : S * T + HIDDEN]
        bo_row = rows_sb[:, S * T + HIDDEN : S * T + 2 * HIDDEN]
        wvo_sb = consts.tile([128, 2 * HC * HIDDEN], BF16, tag="wvo")
        wv_sb = wvo_sb[:, 0 : HC * HIDDEN]
        wo_sb = wvo_sb[:, HC * HIDDEN : 2 * HC * HIDDEN]
        bq_col = consts.tile([HEADS, 1], F32, tag="bqc")
        gb_rep = consts.tile([S, 2 * HIDDEN], F32, tag="gbrep")
        gamma_rep = gb_rep[:, 0:HIDDEN]
        beta_rep = gb_rep[:, HIDDEN : 2 * HIDDEN]
        onesneg = consts.tile([1, HEADS + S], BF16, tag="ones1")
        maskone = onesneg[:, 0:HEADS]  # 1.0: multiplies the pre-scaled mask row
        ones_bf = onesneg[:, HEADS : HEADS + S]
        ones_col = consts.tile([128, 1], BF16, tag="onesc")
        eps_col = consts.tile([S, 1], F32, tag="eps")
        warm = consts.tile([1, 4], F32, tag="warm")
        # Z and 1/Z rows, laid out as col = s*12 + head
        z_row = consts.tile([1, S * HEADS], BF16, tag="zrow")
        zinv_row = consts.tile([1, S * HEADS], BF16, tag="zirow")
        zinv_exp = consts.tile([128, S * HEADS], BF16, tag="ziexp")
        # P^T staging: pt_sb[64h+j, dc*S+s], head(d)=2dc+h, d=head*64+j
        pt_sb = consts.tile([128, HC * S], BF16, tag="pt")
        # Yu^T: 2 tiles of 3 chunks each,
        # yt_sb[i][:, (c%3)*384 + s*12 + head] for c in {3i, 3i+1, 3i+2}
        yt_sb = [
            consts.tile([128, 3 * S * HEADS], BF16, tag=f"yt{i}", name=f"yt{i}")
            for i in range(2)
        ]

        # ---- both HWDGE rings share HBM bandwidth (~370 GB/s total), so
        # every large transfer is split across the two rings and ordered by
        # global priority: consts, b0, b1, Wv, b2, b3, Wo.
        nc.scalar.dma_start(iw_sb[:], identwq[:])
        nc.scalar.dma_start(bq_col[:], bq[:, None])
        nc.scalar.dma_start(rows_sb[:], rows[:])
        nc.scalar.dma_start(gb_rep[:], gb[None, :].broadcast_to((S, 2 * HIDDEN)))

        x_tiles = {}
        for blk in range(NBLK):
            x_tiles[blk] = xp.tile([128, BS * HIDDEN], BF16, tag="xblk", name="x_blk")

        def load_x_half(ring, blk, half):
            s0 = blk * BS + half * 4
            lo = half * 4 * HIDDEN
            if blk == 0 and half == 0:
                for g in range(2):  # 2-sent pieces so the first PE op starts early
                    ring.dma_start(
                        x_tiles[0][:, g * 2 * HIDDEN : (g + 1) * 2 * HIDDEN],
                        hs[2 * g : 2 * g + 2].rearrange("s t h -> t s h"),
                    )
            else:
                ring.dma_start(
                    x_tiles[blk][:, lo : lo + 4 * HIDDEN],
                    hs[s0 : s0 + 4].rearrange("s t h -> t s h"),
                )

        # second halves go through the (otherwise idle) GPSIMD SWDGE ring so
        # their descriptor-gen doesn't block the ACT engine queue
        HH = HC * HIDDEN  # 4608
        for blk in (0, 1):
            load_x_half(nc.sync, blk, 0)
            load_x_half(nc.gpsimd, blk, 1)
        nc.sync.dma_start(wvo_sb[:, 0 : HH // 2], wvo[:, 0 : HH // 2])
        nc.gpsimd.dma_start(wvo_sb[:, HH // 2 : HH], wvo[:, HH // 2 : HH])
        for blk in (2, 3):
            load_x_half(nc.sync, blk, 0)
            load_x_half(nc.gpsimd, blk, 1)
        nc.sync.dma_start(
            wvo_sb[:, HH : HH + HH // 2], wvo[:, HH : HH + HH // 2]
        )
        nc.gpsimd.dma_start(
            wvo_sb[:, HH + HH // 2 : 2 * HH], wvo[:, HH + HH // 2 : 2 * HH]
        )
        # block 2's X^T via the XBAR DMA transpose in the fabric-idle window
        # after the loads; it monopolizes the 16 DMA engines, so only one
        # block goes this way (block 3's transposes stay on the PE).
        # xt layout matches stage_a exactly: xt[p, s*768 + c*128 + t].
        xt_tiles = {}
        xt_tiles[2] = xtp.tile([128, BS * HIDDEN], BF16, tag="xtblk", name="xt_blk")
        nc.sync.dma_start(
            xt_tiles[2][:].rearrange("p (x t) -> p x t", t=128),
            x_tiles[2][:],
            transpose=True,
        )
        xt_tiles[3] = xtp.tile([128, BS * HIDDEN], BF16, tag="xtblk", name="xt_blk")
        nc.scalar.dma_start(
            xt_tiles[3][:].rearrange("p (x t) -> p x t", t=128),
            x_tiles[3][:],
            transpose=True,
        )

        nc.vector.memset(ones_col[:], 1.0)
        nc.vector.memset(onesneg[:], 1.0)
        nc.vector.memset(eps_col[:], LN_EPS)
        nc.vector.memset(warm[:], 1.0)
        # warm the Exp table set before softmax(0) needs it
        nc.scalar.activation(warm[0:1, 2:3], warm[0:1, 0:1], AF.Exp)

        # ---------------- pipeline stages -------------------------------
        def stage_a_sents(blk, xt_blk, sents):
            # X^T slices in SBUF: col = s'*768 + c*128 + t
            x_blk = x_tiles[blk]
            for sp in sents:
                xt_ps = ps_xt.tile([128, HIDDEN], BF16, tag="xtps", name="xt_ps")
                for c in range(HC):
                    nc.tensor.transpose(
                        xt_ps[:, c * 128 : (c + 1) * 128],
                        x_blk[
                            :, sp * HIDDEN + c * 128 : sp * HIDDEN + (c + 1) * 128
                        ],
                        ident_sb[:],
                    )
                eng = nc.scalar.copy if sp % 4 == 3 else nc.vector.tensor_copy
                eng(xt_blk[:, sp * HIDDEN : (sp + 1) * HIDDEN], xt_ps[:])

        def stage_q(blk, xt_blk):
            s0 = blk * BS
            qt_ps = ps_qt.tile([HEADS, BS * T], F32, tag="qt", name="qt_ps")
            xt_r = xt_blk.rearrange("p (s c j) -> p c s j", s=BS, j=128)
            et_sb = smallp.tile([HEADS, BS * T], BF16, tag="et", name="et_sb")
            for c in range(HC):
                for h in range(2):
                    nc.tensor.matmul(
                        qt_ps[:, h * 512 : (h + 1) * 512],
                        wq_sb[:, c * HEADS : (c + 1) * HEADS],
                        xt_r[:, c, h * 4 : (h + 1) * 4],
                        start=(c == 0),
                        stop=False,
                    )
            for h in range(2):
                nc.tensor.matmul(
                    qt_ps[:, h * 512 : (h + 1) * 512],
                    maskone[:],
                    maskneg_row[:, s0 * T + h * 512 : s0 * T + (h + 1) * 512],
                    start=False,
                    stop=True,
                )
                # UNNORMALIZED e^T = exp(q + bq); masked tokens underflow to 0
                nc.scalar.activation(
                    et_sb[:, h * 512 : (h + 1) * 512],
                    qt_ps[:, h * 512 : (h + 1) * 512],
                    AF.Exp,
                    bias=bq_col[:, 0:1],
                )
            return et_sb

        def stage_b_begin(blk):
            e_sb = smallp.tile([128, BS * HEADS], BF16, tag="e", name="e_sb")
            ytp = [
                ps_yt.tile([128, 3 * BS * HEADS], F32, tag="ytps", name=f"ytp{i}")
                for i in range(2)
            ]
            return e_sb, ytp

        def stage_b_half(blk, et_sb, e_sb, ytp, half):
            x_blk = x_tiles[blk]
            ett_ps = ps_xt.tile([128, 4 * HEADS], BF16, tag="xtps", name="ett_ps")
            for hp in range(4):
                sp = half * 4 + hp
                nc.tensor.transpose(
                    ett_ps[:, hp * HEADS : (hp + 1) * HEADS],
                    et_sb[:, sp * T : (sp + 1) * T],
                    ident_sb[0:HEADS, 0:HEADS],
                )
            nc.vector.tensor_copy(
                e_sb[:, half * 4 * HEADS : (half + 1) * 4 * HEADS], ett_ps[:]
            )
            for hp in range(4):
                sp = half * 4 + hp
                for c in range(HC):
                    nc.tensor.matmul(
                        ytp[c // 3][
                            :,
                            (c % 3) * BS * HEADS
                            + sp * HEADS : (c % 3) * BS * HEADS
                            + (sp + 1) * HEADS,
                        ],
                        x_blk[
                            :,
                            sp * HIDDEN + c * 128 : sp * HIDDEN + (c + 1) * 128,
                        ],
                        e_sb[:, sp * HEADS : (sp + 1) * HEADS],
                    )

        def stage_b_fin(blk, e_sb, ytp):
            s0 = blk * BS
            # Z row for this block: Z[s, head] via a K=128 ones matmul
            z_ps = ps_xt.tile([1, BS * HEADS], F32, tag="xtps", name="z_ps")
            nc.tensor.matmul(z_ps[:], ones_col[:], e_sb[:])
            zc = slice(s0 * HEADS, (s0 + BS) * HEADS)
            nc.vector.tensor_copy(z_row[0:1, zc], z_ps[:])
            with nc.allow_low_precision(reason="1/Z in bf16: 0.4% scale err ok"):
                nc.vector.reciprocal(zinv_row[0:1, zc], z_ps[:])
            nc.gpsimd.partition_broadcast(zinv_exp[:, zc], zinv_row[0:1, zc])
            # drain the unnormalized Y^T accumulators
            for i in range(2):
                eng = (nc.vector.tensor_copy, nc.scalar.copy)[i]
                eng(
                    yt_sb[i][:, :].rearrange("p (c sn) -> p c sn", c=3)[
                        :, :, s0 * HEADS : (s0 + BS) * HEADS
                    ],
                    ytp[i].rearrange("p (c sn) -> p c sn", c=3),
                )

        def stage_b(blk, et_sb):
            e_sb, ytp = stage_b_begin(blk)
            stage_b_half(blk, et_sb, e_sb, ytp, 0)
            stage_b_half(blk, et_sb, e_sb, ytp, 1)
            stage_b_fin(blk, e_sb, ytp)

        # G-route: G^T[d, (s,head)] = Wv^T-chunks @ Yu^T + bv ⊗ Z;
        # extract P^T[d, s] = G^T[d, s*12+head(d)] * zinv[s, head(d)]
        def g_route(sent0, nsent):
            c0 = sent0 * HEADS
            c1 = (sent0 + nsent) * HEADS
            zi_r = zinv_exp.rearrange("p (s n) -> p s n", n=HEADS)
            for dc0 in range(0, HC, 2):
                gp = [
                    ps_g.tile([128, nsent * HEADS], F32, tag="g", name="g_ps")
                    for _ in range(2)
                ]
                for c in range(HC):
                    for k in range(2):
                        dc = dc0 + k
                        nc.tensor.matmul(
                            gp[k][:],
                            wv_sb[
                                :,
                                c * HIDDEN + dc * 128 : c * HIDDEN + (dc + 1) * 128,
                            ],
                            yt_sb[c // 3][
                                :,
                                (c % 3) * S * HEADS + c0 : (c % 3) * S * HEADS + c1,
                            ],
                            start=(c == 0),
                            stop=False,
                        )
                for k in range(2):
                    dc = dc0 + k
                    nc.tensor.matmul(
                        gp[k][:],
                        bv_row[0:1, dc * 128 : (dc + 1) * 128],
                        z_row[0:1, c0:c1],
                        start=False,
                        stop=True,
                    )
                for k in range(2):
                    dc = dc0 + k
                    g_r = gp[k].rearrange("p (s n) -> p s n", n=HEADS)
                    for half in range(2):
                        head = 2 * dc + half
                        rows_ = slice(half * 64, half * 64 + 64)
                        nc.vector.tensor_tensor(
                            pt_sb[rows_, dc * S + sent0 : dc * S + sent0 + nsent],
                            g_r[rows_, :, head],
                            zi_r[rows_, sent0 : sent0 + nsent, head],
                            op=ALU.mult,
                        )

        # ---------------- main loop --------------------------------------
        # stage_a's PE transposes don't count as "busy" for the HAM clock
        # gate, so interleave them sent-by-sent with stage_b's real matmuls
        # to keep the PE at 2.4 GHz.  Blocks are processed in ARRIVAL order:
        # 0 (sync, sliced), 3 (scalar, early), 1, 2 (sync).
        order = [0, 1, 2, 3]
        stash = {}
        prev = None
        for i, blk in enumerate(order):
            if blk in xt_tiles:
                xt_blk = xt_tiles[blk]  # transposed by the XBAR DMA
                if prev is not None:
                    et_prev = stash.pop(prev)
                    e_sb, ytp = stage_b_begin(prev)
                    stage_b_half(prev, et_prev, e_sb, ytp, 0)
                    stage_b_half(prev, et_prev, e_sb, ytp, 1)
                    stage_b_fin(prev, e_sb, ytp)
            else:
                xt_blk = xtp.tile(
                    [128, BS * HIDDEN], BF16, tag="xtblk", name="xt_blk"
                )
                if prev is not None:
                    et_prev = stash.pop(prev)
                    e_sb, ytp = stage_b_begin(prev)
                    stage_b_half(prev, et_prev, e_sb, ytp, 0)
                    stage_a_sents(blk, xt_blk, [0, 1])
                    stage_b_half(prev, et_prev, e_sb, ytp, 1)
                    stage_a_sents(blk, xt_blk, [2, 3])
                    stage_b_fin(prev, e_sb, ytp)
                    stage_a_sents(blk, xt_blk, [4, 5, 6, 7])
                else:
                    stage_a_sents(blk, xt_blk, range(BS))
            stash[blk] = stage_q(blk, xt_blk)
            if i == len(order) - 1:
                # switch the ACT table set to sqrt_and_others after the final
                # exp (data-dep on et_sb so the scheduler can't hoist it)
                nc.scalar.activation(
                    warm[0:1, 3:4], stash[blk][0:1, 0:1], AF.Sqrt
                )
            if prev is not None:
                g_route(prev * BS, BS)
            prev = blk
        stage_b(prev, stash.pop(prev))
        g_route(prev * BS, BS)  # last G quarter

        psctx.close()  # free the main-loop PSUM banks

        # ---------------- output projection + layernorm -------------------
        with (
            tc.tile_pool(name="ps_o", bufs=1, space="PSUM") as ps_o,
            tc.tile_pool(name="fin", bufs=1) as fin,
        ):
            # res = P @ Wo + bo   -> [32, 768]
            o1 = ps_o.tile([S, 512], F32, tag="o1", name="o1")
            o2 = ps_o.tile([S, 256], F32, tag="o2", name="o2")
            for dc in range(HC):
                nc.tensor.matmul(
                    o1[:],
                    pt_sb[:, dc * S : (dc + 1) * S],
                    wo_sb[:, dc * HIDDEN : dc * HIDDEN + 512],
                    start=(dc == 0),
                    stop=False,
                )
                nc.tensor.matmul(
                    o2[:],
                    pt_sb[:, dc * S : (dc + 1) * S],
                    wo_sb[:, dc * HIDDEN + 512 : (dc + 1) * HIDDEN],
                    start=(dc == 0),
                    stop=False,
                )
            nc.tensor.matmul(
                o1[:], ones_bf[:], bo_row[:, 0:512], start=False, stop=True
            )
            nc.tensor.matmul(
                o2[:], ones_bf[:], bo_row[:, 512:768], start=False, stop=True
            )

            # mean/var via bn_stats on the two PSUM tiles
            stats = fin.tile([S, 12], F32, tag="stats", name="stats")
            nc.vector.bn_stats(stats[:, 0:6], o1[:])
            nc.vector.bn_stats(stats[:, 6:12], o2[:])
            mv = fin.tile([S, 2], F32, tag="mv", name="mv")
            nc.vector.bn_aggr(mv[:], stats.rearrange("p (g s) -> p g s", g=2))
            # rstd = 1 / sqrt(var + eps); sqrt table pre-warmed in the loop
            sd = fin.tile([S, 1], F32, tag="sd", name="sd")
            nc.scalar.activation(sd[:], mv[:, 1:2], AF.Sqrt, bias=eps_col[:, 0:1])
            rstd = fin.tile([S, 1], F32, tag="rstd", name="rstd")
            nc.vector.reciprocal(rstd[:], sd[:])
            nmr = fin.tile([S, 1], F32, tag="nmr", name="nmr")
            nc.vector.tensor_scalar(
                nmr[:], mv[:, 0:1], rstd[:, 0:1], -1.0, op0=ALU.mult, op1=ALU.mult
            )
            # xn = res * rstd - mu * rstd, read straight from PSUM
            xn = fin.tile([S, HIDDEN], F32, tag="xn", name="xn")
            nc.scalar.activation(
                xn[:, 0:512], o1[:], AF.Identity, bias=nmr[:, 0:1], scale=rstd[:, 0:1]
            )
            nc.vector.tensor_scalar(
                xn[:, 512:768], o2[:], rstd[:, 0:1], nmr[:, 0:1],
                op0=ALU.mult, op1=ALU.add,
            )
            # out = xn * gamma + beta, split DVE / GPSIMD
            t1 = fin.tile([S, HIDDEN], F32, tag="t1", name="t1")
            out_sb = fin.tile([S, HIDDEN], F32, tag="osb", name="out_sb")
            nc.vector.tensor_tensor(
                t1[:, 0:512], xn[:, 0:512], gamma_rep[:, 0:512], op=ALU.mult
            )
            nc.vector.tensor_add(out_sb[:, 0:512], t1[:, 0:512], beta_rep[:, 0:512])
            nc.gpsimd.tensor_tensor(
                t1[:, 512:768], xn[:, 512:768], gamma_rep[:, 512:768], op=ALU.mult
            )
            nc.gpsimd.tensor_tensor(
                out_sb[:, 512:768], t1[:, 512:768], beta_rep[:, 512:768], op=ALU.add
            )
            nc.sync.dma_start(out[:], out_sb[:])


_NC_CACHE = {}


def kernel(hidden_states, mask, Wq, bq, Wv, bv, Wo, bo, gamma, beta):
    if "nc" not in _NC_CACHE:
        _NC_CACHE["nc"] = build_kernel()
    nc = _NC_CACHE["nc"]
    bf16 = ml_dtypes.bfloat16
    f32 = np.float32

    def bf(a):
        return np.ascontiguousarray(np.asarray(a, dtype=f32).astype(bf16))

    def chunked(w):  # [768, N] -> [128, 6*N] with h = c*128 + p
        w = np.asarray(w, dtype=f32)
        n = w.shape[1]
        return w.reshape(HC, 128, n).transpose(1, 0, 2).reshape(128, HC * n)

    identwq = bf(np.concatenate([np.eye(128, dtype=f32), chunked(Wq)], axis=1))
    wvo = bf(np.concatenate([chunked(Wv), chunked(Wo)], axis=1))
    gb = np.ascontiguousarray(np.concatenate([gamma, beta]), dtype=f32)
    bq_c = np.ascontiguousarray(bq, dtype=f32)
    hs_bf = np.asarray(hidden_states, dtype=f32).astype(bf16)
    bvbo = np.concatenate([np.asarray(bv, f32), np.asarray(bo, f32)])

    in_maps = []
    for b in range(N_CORES):
        rows = bf(
            np.concatenate([np.asarray(mask[b], f32).reshape(-1) * MASK_NEG, bvbo])
        )[None, :]
        in_maps.append(
            {
                "hs": np.ascontiguousarray(hs_bf[b]),
                "identwq": identwq,
                "rows": rows,
                "wvo": wvo,
                "bq": bq_c,
                "gb": gb,
            }
        )
    res = run_bass_kernel_spmd(nc, in_maps, core_ids=list(range(N_CORES)))
    _NC_CACHE["last_results"] = res
    globals()["_LAST_RESULTS"] = res
    return np.stack([res.results[i]["out"] for i in range(N_CORES)], axis=0)


# revision 38
# speedup vs baseline: 1.2408x; 1.2408x over previous
"""Trainium2 Bass kernel for nn_MultiHeadSelfTokenAttention.

Reference computation (per (b, s) slice, X = hidden[b, s] in [T=128, H=768]):
    q      = X @ Wq + bq                       [T, 12]     (per-token per-head logit)
    scores = q + mask[:, None] * (-10000)
    alpha  = softmax(scores, axis=T)           [T, 12]
    v      = (X @ Wv + bv).reshape(T, 12, 64)
    res    = einsum('th,thd->hd', alpha, v)    [12, 64] -> [768]
    out    = LN(res @ Wo + bo) * gamma + beta  [768]

Key algebraic restructure: with e = exp(scores) (unnormalized) and Z = sum_t e,
    Yu[head, h]  = sum_t e[t, head] * X[t, h]          (unnormalized pool)
    G[d, s]      = (Yu[head(d)] @ Wv[:, d]) + bv[d] * Z[s, head(d)]
    P[d, s]      = G[d, s] / Z[s, head(d)]
so V is never materialized AND the softmax normalization folds into the
G-route extract (one tensor_tensor with a partition-broadcast 1/Z).

v6 (on top of v5):
  - all bf16 operands (X, Wq, Wv, Wo, mask, ident, bv, bo) are pre-cast and
    pre-laid-out on the HOST: HBM traffic drops 17.4MB -> 8.8MB per core,
    every load runs on the two HWDGE rings (no SWDGE cast DMA, no on-chip
    weight cast passes), and the gpsimd engine only does the 1/Z broadcast.
  - X blocks land every ~5-7us so the PE never starves early and the HAM
    clock gate stays at 2.4 GHz.
  - ps_g double-buffered (G-route dc iterations pipeline); ett/z share the
    ps_xt slots.
  - g_route quarter per iteration right after stage_q (fills the exp
    round-trip); sqrt-table warm is data-dependent on the last exp so the
    scheduler cannot hoist it before.
  - LN tail: eps folded into Sqrt bias, xn split ACT/DVE, gamma/beta apply
    split DVE/gpsimd.

Sharding: data-parallel across batch; core b handles hidden_states[b]
(32 sents).  Weights replicated.  No collectives.
"""

import os
import sys
from contextlib import ExitStack

import numpy as np

for _p in ("/opt/trn_rl_repo", "/root/.axon_site/_ro/trn_rl_repo"):
    if os.path.isdir(_p) and _p not in sys.path:
        sys.path.insert(0, _p)

import ml_dtypes

import concourse.bacc as bacc
import concourse.bass as bass
import concourse.bass_utils as _bu
import concourse.tile as tile
from concourse import mybir
from concourse.bass_utils import run_bass_kernel_spmd



F32 = mybir.dt.float32
BF16 = mybir.dt.bfloat16
AF = mybir.ActivationFunctionType
ALU = mybir.AluOpType

HIDDEN = 768
HEADS = 12
B, S, T = 8, 32, 128
HC = HIDDEN // 128  # 6 chunks of the hidden dim
LN_EPS = 1e-5
MASK_NEG = -10000.0
N_CORES = 8
BS = 8  # sents per block
NBLK = S // BS


def build_kernel():
    nc = bacc.Bacc(trn_type="TRN2", target_bir_lowering=False, debug=False)

    # all-bf16 device inputs, pre-laid-out on the host
    hs = nc.dram_tensor("hs", [S, T, HIDDEN], BF16, kind="ExternalInput").ap()
    # identwq[128, 0:128] = I128, [128, 128:200] = Wq chunked (h = c*128 + p)
    identwq = nc.dram_tensor(
        "identwq", [128, 128 + HC * HEADS], BF16, kind="ExternalInput"
    ).ap()
    # rows[0, :]: mask*(-1e4) flattened (4096) | bv (768) | bo (768)
    rows = nc.dram_tensor(
        "rows", [1, S * T + 2 * HIDDEN], BF16, kind="ExternalInput"
    ).ap()
    # wvo[128, 0:4608] = Wv chunked, [128, 4608:9216] = Wo chunked
    wvo = nc.dram_tensor(
        "wvo", [128, 2 * HC * HIDDEN], BF16, kind="ExternalInput"
    ).ap()
    bq = nc.dram_tensor("bq", [HEADS], F32, kind="ExternalInput").ap()
    gb = nc.dram_tensor("gb", [2 * HIDDEN], F32, kind="ExternalInput").ap()
    out = nc.dram_tensor("out", [S, HIDDEN], F32, kind="ExternalOutput").ap()

    with tile.TileContext(nc) as tc:
        kernel_body(tc, out, hs, identwq, rows, wvo, bq, gb)
    nc.compile()
    return nc


def kernel_body(tc, out, hs, identwq, rows, wvo, bq, gb):
    nc = tc.nc
    with ExitStack() as ctx:
        consts = ctx.enter_context(tc.tile_pool(name="consts", bufs=1))
        xp = ctx.enter_context(tc.tile_pool(name="x", bufs=4))
        xtp = ctx.enter_context(tc.tile_pool(name="xt", bufs=4))
        smallp = ctx.enter_context(tc.tile_pool(name="small", bufs=2))
        psctx = ExitStack()
        ps_xt = psctx.enter_context(tc.tile_pool(name="ps_xt", bufs=2, space="PSUM"))
        ps_qt = psctx.enter_context(tc.tile_pool(name="ps_qt", bufs=1, space="PSUM"))
        ps_yt = psctx.enter_context(tc.tile_pool(name="ps_yt", bufs=2, space="PSUM"))
        ps_g = psctx.enter_context(tc.tile_pool(name="ps_g", bufs=2, space="PSUM"))

        # ---- tiles ----
        iw_sb = consts.tile([128, 128 + HC * HEADS], BF16, tag="iw")
        ident_sb = iw_sb[:, 0:128]
        wq_sb = iw_sb[:, 128 : 128 + HC * HEADS]
        rows_sb = consts.tile([1, S * T + 2 * HIDDEN], BF16, tag="rows")
        maskneg_row = rows_sb[:, 0 : S * T]  # already scaled by -1e4 on host
        bv_row = rows_sb[:, S * T : S * T + HIDDEN]
        bo_row = rows_sb[:, S * T + HIDDEN : S * T + 2 * HIDDEN]
        wvo_sb = consts.tile([128, 2 * HC * HIDDEN], BF16, tag="wvo")
        wv_sb = wvo_sb[:, 0 : HC * HIDDEN]
        wo_sb = wvo_sb[:, HC * HIDDEN : 2 * HC * HIDDEN]
        bq_col = consts.tile([HEADS, 1], F32, tag="bqc")
        gb_rep = consts.tile([S, 2 * HIDDEN], F32, tag="gbrep")
        gamma_rep = gb_rep[:, 0:HIDDEN]
        beta_rep = gb_rep[:, HIDDEN : 2 * HIDDEN]
        onesneg = consts.tile([1, HEADS + S], BF16, tag="ones1")
        maskone = onesneg[:, 0:HEADS]  # 1.0: multiplies the pre-scaled mask row
        ones_bf = onesneg[:, HEADS : HEADS + S]
        ones_col = consts.tile([128, 1], BF16, tag="onesc")
        eps_col = consts.tile([S, 1], F32, tag="eps")
        warm = consts.tile([1, 4], F32, tag="warm")
        # Z and 1/Z rows, laid out as col = s*12 + head
        z_row = consts.tile([1, S * HEADS], BF16, tag="zrow")
        zinv_row = consts.tile([1, S * HEADS], BF16, tag="zirow")
        zinv_exp = consts.tile([128, S * HEADS], BF16, tag="ziexp")
        # P^T staging: pt_sb[64h+j, dc*S+s], head(d)=2dc+h, d=head*64+j
        pt_sb = consts.tile([128, HC * S], BF16, tag="pt")
        # Yu^T: 2 tiles of 3 chunks each,
        # yt_sb[i][:, (c%3)*384 + s*12 + head] for c in {3i, 3i+1, 3i+2}
        yt_sb = [
            consts.tile([128, 3 * S * HEADS], BF16, tag=f"yt{i}", name=f"yt{i}")
            for i in range(2)
        ]

        # ---- both HWDGE rings share HBM bandwidth (~370 GB/s total), so
        # every large transfer is split across the two rings and ordered by
        # global priority: consts, b0, b1, Wv, b2, b3, Wo.
        nc.scalar.dma_start(iw_sb[:], identwq[:])
        nc.scalar.dma_start(bq_col[:], bq[:, None])
        nc.scalar.dma_start(rows_sb[:], rows[:])
        nc.scalar.dma_start(gb_rep[:], gb[None, :].broadcast_to((S, 2 * HIDDEN)))

        x_tiles = {}
        for blk in range(NBLK):
            x_tiles[blk] = xp.tile([128, BS * HIDDEN], BF16, tag="xblk", name="x_blk")

        def load_x_half(ring, blk, half):
            s0 = blk * BS + half * 4
            lo = half * 4 * HIDDEN
            if blk == 0 and half == 0:
                for g in range(2):  # 2-sent pieces so the first PE op starts early
                    ring.dma_start(
                        x_tiles[0][:, g * 2 * HIDDEN : (g + 1) * 2 * HIDDEN],
                        hs[2 * g : 2 * g + 2].rearrange("s t h -> t s h"),
                    )
            else:
                ring.dma_start(
                    x_tiles[blk][:, lo : lo + 4 * HIDDEN],
                    hs[s0 : s0 + 4].rearrange("s t h -> t s h"),
                )

        # second halves go through the (otherwise idle) GPSIMD SWDGE ring so
        # their descriptor-gen doesn't block the ACT engine queue
        HH = HC * HIDDEN  # 4608
        for blk in (0, 1):
            load_x_half(nc.sync, blk, 0)
            load_x_half(nc.gpsimd, blk, 1)
        nc.sync.dma_start(wvo_sb[:, 0 : HH // 2], wvo[:, 0 : HH // 2])
        nc.gpsimd.dma_start(wvo_sb[:, HH // 2 : HH], wvo[:, HH // 2 : HH])
        for blk in (2, 3):
            load_x_half(nc.sync, blk, 0)
            load_x_half(nc.gpsimd, blk, 1)
        nc.sync.dma_start(
            wvo_sb[:, HH : HH + HH // 2], wvo[:, HH : HH + HH // 2]
        )
        nc.gpsimd.dma_start(
            wvo_sb[:, HH + HH // 2 : 2 * HH], wvo[:, HH + HH // 2 : 2 * HH]
        )
        # block 2's X^T via the XBAR DMA transpose in the fabric-idle window
        # after the loads; it monopolizes the 16 DMA engines, so only one
        # block goes this way (block 3's transposes stay on the PE).
        # xt layout matches stage_a exactly: xt[p, s*768 + c*128 + t].
        xt_tiles = {}
        xt_tiles[2] = xtp.tile([128, BS * HIDDEN], BF16, tag="xtblk", name="xt_blk")
        nc.sync.dma_start(
            xt_tiles[2][:].rearrange("p (x t) -> p x t", t=128),
            x_tiles[2][:],
            transpose=True,
        )


        nc.vector.memset(ones_col[:], 1.0)
        nc.vector.memset(onesneg[:], 1.0)
        nc.vector.memset(eps_col[:], LN_EPS)
        nc.vector.memset(warm[:], 1.0)
        # warm the Exp table set before softmax(0) needs it
        nc.scalar.activation(warm[0:1, 2:3], warm[0:1, 0:1], AF.Exp)

        # ---------------- pipeline stages -------------------------------
        def stage_a_sents(blk, xt_blk, sents):
            # X^T slices in SBUF: col = s'*768 + c*128 + t
            x_blk = x_tiles[blk]
            for sp in sents:
                xt_ps = ps_xt.tile([128, HIDDEN], BF16, tag="xtps", name="xt_ps")
                for c in range(HC):
                    nc.tensor.transpose(
                        xt_ps[:, c * 128 : (c + 1) * 128],
                        x_blk[
                            :, sp * HIDDEN + c * 128 : sp * HIDDEN + (c + 1) * 128
                        ],
                        ident_sb[:],
                    )
                eng = nc.scalar.copy if sp % 4 == 3 else nc.vector.tensor_copy
                eng(xt_blk[:, sp * HIDDEN : (sp + 1) * HIDDEN], xt_ps[:])

        def stage_q(blk, xt_blk):
            s0 = blk * BS
            qt_ps = ps_qt.tile([HEADS, BS * T], F32, tag="qt", name="qt_ps")
            xt_r = xt_blk.rearrange("p (s c j) -> p c s j", s=BS, j=128)
            et_sb = smallp.tile([HEADS, BS * T], BF16, tag="et", name="et_sb")
            for c in range(HC):
                for h in range(2):
                    nc.tensor.matmul(
                        qt_ps[:, h * 512 : (h + 1) * 512],
                        wq_sb[:, c * HEADS : (c + 1) * HEADS],
                        xt_r[:, c, h * 4 : (h + 1) * 4],
                        start=(c == 0),
                        stop=False,
                    )
            for h in range(2):
                nc.tensor.matmul(
                    qt_ps[:, h * 512 : (h + 1) * 512],
                    maskone[:],
                    maskneg_row[:, s0 * T + h * 512 : s0 * T + (h + 1) * 512],
                    start=False,
                    stop=True,
                )
                # UNNORMALIZED e^T = exp(q + bq); masked tokens underflow to 0
                nc.scalar.activation(
                    et_sb[:, h * 512 : (h + 1) * 512],
                    qt_ps[:, h * 512 : (h + 1) * 512],
                    AF.Exp,
                    bias=bq_col[:, 0:1],
                )
            return et_sb

        def stage_b_begin(blk):
            e_sb = smallp.tile([128, BS * HEADS], BF16, tag="e", name="e_sb")
            ytp = [
                ps_yt.tile([128, 3 * BS * HEADS], F32, tag="ytps", name=f"ytp{i}")
                for i in range(2)
            ]
            return e_sb, ytp

        def stage_b_half(blk, et_sb, e_sb, ytp, half):
            x_blk = x_tiles[blk]
            ett_ps = ps_xt.tile([128, 4 * HEADS], BF16, tag="xtps", name="ett_ps")
            for hp in range(4):
                sp = half * 4 + hp
                nc.tensor.transpose(
                    ett_ps[:, hp * HEADS : (hp + 1) * HEADS],
                    et_sb[:, sp * T : (sp + 1) * T],
                    ident_sb[0:HEADS, 0:HEADS],
                )
            nc.vector.tensor_copy(
                e_sb[:, half * 4 * HEADS : (half + 1) * 4 * HEADS], ett_ps[:]
            )
            for hp in range(4):
                sp = half * 4 + hp
                for c in range(HC):
                    nc.tensor.matmul(
                        ytp[c // 3][
                            :,
                            (c % 3) * BS * HEADS
                            + sp * HEADS : (c % 3) * BS * HEADS
                            + (sp + 1) * HEADS,
                        ],
                        x_blk[
                            :,
                            sp * HIDDEN + c * 128 : sp * HIDDEN + (c + 1) * 128,
                        ],
                        e_sb[:, sp * HEADS : (sp + 1) * HEADS],
                    )

        def stage_b_fin(blk, e_sb, ytp):
            s0 = blk * BS
            # Z row for this block: Z[s, head] via a K=128 ones matmul
            z_ps = ps_xt.tile([1, BS * HEADS], F32, tag="xtps", name="z_ps")
            nc.tensor.matmul(z_ps[:], ones_col[:], e_sb[:])
            zc = slice(s0 * HEADS, (s0 + BS) * HEADS)
            nc.vector.tensor_copy(z_row[0:1, zc], z_ps[:])
            with nc.allow_low_precision(reason="1/Z in bf16: 0.4% scale err ok"):
                nc.vector.reciprocal(zinv_row[0:1, zc], z_ps[:])
            nc.gpsimd.partition_broadcast(zinv_exp[:, zc], zinv_row[0:1, zc])
            # drain the unnormalized Y^T accumulators
            for i in range(2):
                eng = (nc.vector.tensor_copy, nc.scalar.copy)[i]
                eng(
                    yt_sb[i][:, :].rearrange("p (c sn) -> p c sn", c=3)[
                        :, :, s0 * HEADS : (s0 + BS) * HEADS
                    ],
                    ytp[i].rearrange("p (c sn) -> p c sn", c=3),
                )

        def stage_b(blk, et_sb):
            e_sb, ytp = stage_b_begin(blk)
            stage_b_half(blk, et_sb, e_sb, ytp, 0)
            stage_b_half(blk, et_sb, e_sb, ytp, 1)
            stage_b_fin(blk, e_sb, ytp)

        # G-route: G^T[d, (s,head)] = Wv^T-chunks @ Yu^T + bv ⊗ Z;
        # extract P^T[d, s] = G^T[d, s*12+head(d)] * zinv[s, head(d)]
        def g_route(sent0, nsent):
            c0 = sent0 * HEADS
            c1 = (sent0 + nsent) * HEADS
            zi_r = zinv_exp.rearrange("p (s n) -> p s n", n=HEADS)
            for dc0 in range(0, HC, 2):
                gp = [
                    ps_g.tile([128, nsent * HEADS], F32, tag="g", name="g_ps")
                    for _ in range(2)
                ]
                for c in range(HC):
                    for k in range(2):
                        dc = dc0 + k
                        nc.tensor.matmul(
                            gp[k][:],
                            wv_sb[
                                :,
                                c * HIDDEN + dc * 128 : c * HIDDEN + (dc + 1) * 128,
                            ],
                            yt_sb[c // 3][
                                :,
                                (c % 3) * S * HEADS + c0 : (c % 3) * S * HEADS + c1,
                            ],
                            start=(c == 0),
                            stop=False,
                        )
                for k in range(2):
                    dc = dc0 + k
                    nc.tensor.matmul(
                        gp[k][:],
                        bv_row[0:1, dc * 128 : (dc + 1) * 128],
                        z_row[0:1, c0:c1],
                        start=False,
                        stop=True,
                    )
                for k in range(2):
                    dc = dc0 + k
                    g_r = gp[k].rearrange("p (s n) -> p s n", n=HEADS)
                    for half in range(2):
                        head = 2 * dc + half
                        rows_ = slice(half * 64, half * 64 + 64)
                        nc.vector.tensor_tensor(
                            pt_sb[rows_, dc * S + sent0 : dc * S + sent0 + nsent],
                            g_r[rows_, :, head],
                            zi_r[rows_, sent0 : sent0 + nsent, head],
                            op=ALU.mult,
                        )

        # ---------------- main loop --------------------------------------
        # stage_a's PE transposes don't count as "busy" for the HAM clock
        # gate, so interleave them sent-by-sent with stage_b's real matmuls
        # to keep the PE at 2.4 GHz.  Blocks are processed in ARRIVAL order:
        # 0 (sync, sliced), 3 (scalar, early), 1, 2 (sync).
        order = [0, 1, 2, 3]
        stash = {}
        prev = None
        for i, blk in enumerate(order):
            if blk in xt_tiles:
                xt_blk = xt_tiles[blk]  # transposed by the XBAR DMA
                if prev is not None:
                    et_prev = stash.pop(prev)
                    e_sb, ytp = stage_b_begin(prev)
                    stage_b_half(prev, et_prev, e_sb, ytp, 0)
                    stage_b_half(prev, et_prev, e_sb, ytp, 1)
                    stage_b_fin(prev, e_sb, ytp)
            else:
                xt_blk = xtp.tile(
                    [128, BS * HIDDEN], BF16, tag="xtblk", name="xt_blk"
                )
                if prev is not None:
                    et_prev = stash.pop(prev)
                    e_sb, ytp = stage_b_begin(prev)
                    stage_b_half(prev, et_prev, e_sb, ytp, 0)
                    stage_a_sents(blk, xt_blk, [0, 1])
                    stage_b_half(prev, et_prev, e_sb, ytp, 1)
                    stage_a_sents(blk, xt_blk, [2, 3])
                    stage_b_fin(prev, e_sb, ytp)
                    stage_a_sents(blk, xt_blk, [4, 5, 6, 7])
                else:
                    stage_a_sents(blk, xt_blk, range(BS))
            stash[blk] = stage_q(blk, xt_blk)
            if i == len(order) - 1:
                # switch the ACT table set to sqrt_and_others after the final
                # exp (data-dep on et_sb so the scheduler can't hoist it)
                nc.scalar.activation(
                    warm[0:1, 3:4], stash[blk][0:1, 0:1], AF.Sqrt
                )
            if prev is not None:
                g_route(prev * BS, BS)
            prev = blk
        stage_b(prev, stash.pop(prev))
        g_route(prev * BS, BS)  # last G quarter

        psctx.close()  # free the main-loop PSUM banks

        # ---------------- output projection + layernorm -------------------
        with (
            tc.tile_pool(name="ps_o", bufs=1, space="PSUM") as ps_o,
            tc.tile_pool(name="fin", bufs=1) as fin,
        ):
            # res = P @ Wo + bo   -> [32, 768]
            o1 = ps_o.tile([S, 512], F32, tag="o1", name="o1")
            o2 = ps_o.tile([S, 256], F32, tag="o2", name="o2")
            for dc in range(HC):
                nc.tensor.matmul(
                    o1[:],
                    pt_sb[:, dc * S : (dc + 1) * S],
                    wo_sb[:, dc * HIDDEN : dc * HIDDEN + 512],
                    start=(dc == 0),
                    stop=False,
                )
                nc.tensor.matmul(
                    o2[:],
                    pt_sb[:, dc * S : (dc + 1) * S],
                    wo_sb[:, dc * HIDDEN + 512 : (dc + 1) * HIDDEN],
                    start=(dc == 0),
                    stop=False,
                )
            nc.tensor.matmul(
                o1[:], ones_bf[:], bo_row[:, 0:512], start=False, stop=True
            )
            nc.tensor.matmul(
                o2[:], ones_bf[:], bo_row[:, 512:768], start=False, stop=True
            )

            # mean/var via bn_stats on the two PSUM tiles
            stats = fin.tile([S, 12], F32, tag="stats", name="stats")
            nc.vector.bn_stats(stats[:, 0:6], o1[:])
            nc.vector.bn_stats(stats[:, 6:12], o2[:])
            mv = fin.tile([S, 2], F32, tag="mv", name="mv")
            nc.vector.bn_aggr(mv[:], stats.rearrange("p (g s) -> p g s", g=2))
            # rstd = 1 / sqrt(var + eps); sqrt table pre-warmed in the loop
            sd = fin.tile([S, 1], F32, tag="sd", name="sd")
            nc.scalar.activation(sd[:], mv[:, 1:2], AF.Sqrt, bias=eps_col[:, 0:1])
            rstd = fin.tile([S, 1], F32, tag="rstd", name="rstd")
            nc.vector.reciprocal(rstd[:], sd[:])
            nmr = fin.tile([S, 1], F32, tag="nmr", name="nmr")
            nc.vector.tensor_scalar(
                nmr[:], mv[:, 0:1], rstd[:, 0:1], -1.0, op0=ALU.mult, op1=ALU.mult
            )
            # xn = res * rstd - mu * rstd, read straight from PSUM
            xn = fin.tile([S, HIDDEN], F32, tag="xn", name="xn")
            nc.scalar.activation(
                xn[:, 0:512], o1[:], AF.Identity, bias=nmr[:, 0:1], scale=rstd[:, 0:1]
            )
            nc.vector.tensor_scalar(
                xn[:, 512:768], o2[:], rstd[:, 0:1], nmr[:, 0:1],
                op0=ALU.mult, op1=ALU.add,
            )
            # out = xn * gamma + beta, split DVE / GPSIMD
            t1 = fin.tile([S, HIDDEN], F32, tag="t1", name="t1")
            out_sb = fin.tile([S, HIDDEN], F32, tag="osb", name="out_sb")
            nc.vector.tensor_tensor(
                t1[:, 0:512], xn[:, 0:512], gamma_rep[:, 0:512], op=ALU.mult
            )
            nc.vector.tensor_add(out_sb[:, 0:512], t1[:, 0:512], beta_rep[:, 0:512])
            nc.gpsimd.tensor_tensor(
                t1[:, 512:768], xn[:, 512:768], gamma_rep[:, 512:768], op=ALU.mult
            )
            nc.gpsimd.tensor_tensor(
                out_sb[:, 512:768], t1[:, 512:768], beta_rep[:, 512:768], op=ALU.add
            )
            nc.sync.dma_start(out[:], out_sb[:])


_NC_CACHE = {}


def kernel(hidden_states, mask, Wq, bq, Wv, bv, Wo, bo, gamma, beta):
    if "nc" not in _NC_CACHE:
        _NC_CACHE["nc"] = build_kernel()
    nc = _NC_CACHE["nc"]
    bf16 = ml_dtypes.bfloat16
    f32 = np.float32

    def bf(a):
        return np.ascontiguousarray(np.asarray(a, dtype=f32).astype(bf16))

    def chunked(w):  # [768, N] -> [128, 6*N] with h = c*128 + p
        w = np.asarray(w, dtype=f32)
        n = w.shape[1]
        return w.reshape(HC, 128, n).transpose(1, 0, 2).reshape(128, HC * n)

    identwq = bf(np.concatenate([np.eye(128, dtype=f32), chunked(Wq)], axis=1))
    wvo = bf(np.concatenate([chunked(Wv), chunked(Wo)], axis=1))
    gb = np.ascontiguousarray(np.concatenate([gamma, beta]), dtype=f32)
    bq_c = np.ascontiguousarray(bq, dtype=f32)
    hs_bf = np.asarray(hidden_states, dtype=f32).astype(bf16)
    bvbo = np.concatenate([np.asarray(bv, f32), np.asarray(bo, f32)])

    in_maps = []
    for b in range(N_CORES):
        rows = bf(
            np.concatenate([np.asarray(mask[b], f32).reshape(-1) * MASK_NEG, bvbo])
        )[None, :]
        in_maps.append(
            {
                "hs": np.ascontiguousarray(hs_bf[b]),
                "identwq": identwq,
                "rows": rows,
                "wvo": wvo,
                "bq": bq_c,
                "gb": gb,
            }
        )
    res = run_bass_kernel_spmd(nc, in_maps, core_ids=list(range(N_CORES)))
    _NC_CACHE["last_results"] = res
    globals()["_LAST_RESULTS"] = res
    return np.stack([res.results[i]["out"] for i in range(N_CORES)], axis=0)
